# revision 10
# baseline (speedup 1.0000x reference)
"""Trainium2 Bass kernel for nn_DilatedGraphConvolutionCell (8-core SPMD).

- Dead-code elimination: output = [Z0..Z3 at t=32] transitively needs only U
  columns {26..32,0,1}, conv at Z0:{28..32} Z1:{30,32} Z2:{32} Z3:{32}, and
  15 real FC rows + one shared fc(0) row.
- FC weights output-sharded 8 ways, fp32->fp16 cast-DMA, SBUF-resident;
  W-stationary matmuls give feature-on-partition outputs.
- Adjacency node-sharded; S computed transposed (softmax via ones-matmuls,
  no cross-partition reductions). A^T cached fp16 for all 25 pairs, reused by
  all 4 layers. Degree normalization (==1.0 +- 1e-7) skipped.
- All DMA patterns keep big partition strides + contiguous inner runs
  (transpose-style partition-stride-1 patterns are descriptor bombs).
  Where a transpose is unavoidable (conv Z -> next FC input) it's done on the
  PE via identity-matmul on [32,128] blocks.
"""
import numpy as np
from contextlib import ExitStack

import concourse.bass as bass
import concourse.tile as tile
from concourse import bacc, mybir
from concourse.bass_utils import run_bass_kernel_spmd
from concourse.masks import make_identity

F32 = mybir.dt.float32
F16 = mybir.dt.float16

NC = 8
N = 500
L = 33
FE = 128
DD = 64
DO = 64
FC1W = 1024
FC2W = 1024
KTF = 18000
NODES_PER_CORE = 64
REAL_NODES = [64] * 7 + [52]
NODE0 = [64 * c for c in range(NC)]

T9 = [26, 27, 28, 29, 30, 31, 32, 0, 1]
T9IDX = {t: i for i, t in enumerate(T9)}
T5 = [28, 29, 30, 31, 32]
PAIRS = []
PAIR_ID = {}
for _t in T5:
    for _d in range(-2, 3):
        _p = ((_t + _d) % L, _t)
        if _p not in PAIR_ID:
            PAIR_ID[_p] = len(PAIRS)
            PAIRS.append(_p)

CONV_TS = [[28, 29, 30, 31, 32], [30, 32], [32], [32]]
R_PASS = [8, 5, 2, 1]
XROW = {
    0: {t: (0, t - 26) for t in range(26, 33)},
    1: {t: (1, t - 28) for t in range(28, 33)},
    2: {30: (2, 0), 31: (0, 7), 32: (2, 1)},
    3: {30: (0, 7), 31: (0, 7), 32: (3, 0)},
}
MCH = [(0, 128), (128, 128), (256, 128), (384, 116)]
W3RES_J = 44
KT1 = 250
RG = [list(range(NC))]
RELU = mybir.ActivationFunctionType.Relu
EXP = mybir.ActivationFunctionType.Exp


def dap(handle, off, dims):
    """Custom AP: dims = [(step_elems, count), ...]; first dim = partitions."""
    t = handle.tensor if isinstance(handle, bass.AP) else handle
    base = handle.offset if isinstance(handle, bass.AP) else 0
    return bass.AP(tensor=t, offset=base + off, ap=[[s, n] for s, n in dims])


def rap(ap_obj, dims):
    """AP on same tensor as ap_obj with custom free dims (keeps partitions)."""
    return bass.AP(tensor=ap_obj.tensor, offset=ap_obj.offset,
                   ap=[list(ap_obj.ap[0])] + [[s, n] for s, n in dims])


def build(debug=False):
    nc = bacc.Bacc("TRN2", target_bir_lowering=False, debug=False,
                   num_devices=NC)

    def inp(name, shape):
        return nc.declare_dram_parameter(name, list(shape), F32, isOutput=False)

    li = inp("li", (N, NODES_PER_CORE, 9))     # host pre-T: [m, n_own, t]
    tfs = inp("tfs", (KTF // NC, 9))           # host pre-T: [k_own, t]
    obs7t = inp("obs7t", (128, 7, KT1))        # host pre-T: [k%128, r, kt]
    ws1 = inp("ws1", (N, 256))
    bs1 = inp("bs1", (256,))
    ws2 = inp("ws2", (256, FE))
    bs2 = inp("bs2", (FE,))
    wt1s = inp("wt1s", (KTF // NC, 256))
    bt1 = inp("bt1", (256,))
    wt2 = inp("wt2", (256, FE))
    bt2 = inp("bt2", (FE,))
    bmat = inp("bmat", (FE, FE))
    w1s = inp("w1s", (N * DD, 128))
    b1s = inp("b1s", (128,))
    w2s = inp("w2s", (FC1W, 128))
    b2s = inp("b2s", (128,))
    w3s = inp("w3s", (FC2W, 8192))
    b3st = inp("b3st", (128, 64))              # host pre-T: [p, j]
    wfb = inp("wfb", (5, FE, DO))
    bconv = inp("bconv", (DO,))

    out_ext = nc.declare_dram_parameter(
        "out", [4, NODES_PER_CORE, DO], F32, isOutput=True)
    dbg = {}
    if debug:
        dbg["dbg_u"] = nc.declare_dram_parameter(
            "dbg_u", [NC * 128, 576], F32, isOutput=True)
        dbg["dbg_x0"] = nc.declare_dram_parameter(
            "dbg_x0", [NC * 128, 8 * 64], F32, isOutput=True)
        dbg["dbg_h1"] = nc.declare_dram_parameter(
            "dbg_h1", [FC1W, 8], F32, isOutput=True)
        dbg["dbg_at"] = nc.declare_dram_parameter(
            "dbg_at", [128, 25 * 4 * 64], F32, isOutput=True)

    with ExitStack() as ctx:
        tc = ctx.enter_context(tile.TileContext(nc))
        pw = ctx.enter_context(tc.tile_pool(name="pw", bufs=1))
        dram = ctx.enter_context(tc.tile_pool(name="dram", bufs=1, space="DRAM"))

        ones_c = pw.tile([128, 1], F32)
        nc.vector.memset(ones_c, 1.0)
        ones_r = pw.tile([1, 128], F32)
        nc.vector.memset(ones_r, 1.0)
        ident = pw.tile([128, 128], F16)
        make_identity(nc, ident)
        b1_sb = pw.tile([128, 1], F32)
        nc.gpsimd.dma_start(out=b1_sb, in_=dap(b1s, 0, [(1, 128), (0, 1)]))
        b2_sb = pw.tile([128, 1], F32)
        nc.gpsimd.dma_start(out=b2_sb, in_=dap(b2s, 0, [(1, 128), (0, 1)]))
        b3_sb = pw.tile([128, 64], F32)
        nc.gpsimd.dma_start(out=b3_sb, in_=dap(b3st, 0, [(64, 128), (1, 64)]))
        bcb_sb = pw.tile([64, 64], F32)        # bconv broadcast over nodes
        nc.gpsimd.dma_start(out=bcb_sb, in_=dap(bconv, 0, [(0, 64), (1, 64)]))
        wfb_sb = pw.tile([128, 5, 64], F16)
        nc.gpsimd.dma_start(
            out=wfb_sb, in_=dap(wfb, 0, [(64, 128), (128 * 64, 5), (1, 64)]))

        w1_sb = pw.tile([128, KT1, 128], F16)
        for k0, kn in [(0, 125), (125, 125)]:
            nc.gpsimd.dma_start(
                out=w1_sb[:, k0:k0 + kn, :],
                in_=dap(w1s, k0 * 128 * 128,
                        [(128, 128), (128 * 128, kn), (1, 128)]))
        w2_sb = pw.tile([128, 8, 128], F16)
        nc.gpsimd.dma_start(
            out=w2_sb, in_=dap(w2s, 0, [(128, 128), (128 * 128, 8), (1, 128)]))

        at_sb = pw.tile([128, 25, 4, 64], F16)
        zrow_sb = pw.tile([128, 8, 64], F16)

        # =============== U phase + adjacency ===============
        with tc.tile_pool(name="pu", bufs=1) as pu, \
             tc.tile_pool(name="pue", bufs=3) as pue, \
             tc.tile_pool(name="ppsu", bufs=1, space="PSUM") as ppsu:
            liT = pu.tile([128, 4, 64, 9], F32)
            nc.gpsimd.dma_start(
                out=liT[:125].rearrange("p mt n t -> p mt (n t)"),
                in_=dap(li, 0, [(576, 125), (125 * 576, 4), (1, 576)]))
            ws1_sb = pu.tile([128, 4, 2, 128], F32)
            for mt in range(4):
                nc.gpsimd.dma_start(
                    out=ws1_sb[:125, mt],
                    in_=dap(ws1, mt * 125 * 256,
                            [(256, 125), (128, 2), (1, 128)]))
            ws2_sb = pu.tile([128, 2, 128], F32)
            nc.gpsimd.dma_start(
                out=ws2_sb, in_=dap(ws2, 0, [(128, 128), (128 * 128, 2), (1, 128)]))
            bs1_sb = pu.tile([128, 2], F32)
            nc.gpsimd.dma_start(out=bs1_sb, in_=dap(bs1, 0, [(1, 128), (128, 2)]))
            bs2_sb = pu.tile([128, 1], F32)
            nc.gpsimd.dma_start(out=bs2_sb, in_=dap(bs2, 0, [(1, 128), (0, 1)]))
            b_sb = pu.tile([128, 128], F32)
            nc.gpsimd.dma_start(out=b_sb, in_=dap(bmat, 0, [(128, 128), (1, 128)]))
            tfT = pu.tile([128, 18, 9], F32)
            nc.gpsimd.dma_start(
                out=tfT[:125],
                in_=dap(tfs, 0, [(9, 125), (125 * 9, 18), (1, 9)]))
            wt1_sb = pu.tile([128, 18, 2, 128], F32)
            for kt in range(18):
                nc.gpsimd.dma_start(
                    out=wt1_sb[:125, kt],
                    in_=dap(wt1s, kt * 125 * 256,
                            [(256, 125), (128, 2), (1, 128)]))
            bt1_sb = pu.tile([128, 2], F32)
            nc.gpsimd.dma_start(out=bt1_sb, in_=dap(bt1, 0, [(1, 128), (128, 2)]))
            wt2_sb = pu.tile([128, 2, 128], F32)
            nc.gpsimd.dma_start(
                out=wt2_sb, in_=dap(wt2, 0, [(128, 128), (128 * 128, 2), (1, 128)]))
            bt2_sb = pu.tile([128, 1], F32)
            nc.gpsimd.dma_start(out=bt2_sb, in_=dap(bt2, 0, [(1, 128), (0, 1)]))

            # temporal MLP layer 1 partial + AllReduce
            ut1p = pu.tile([128, 2, 9], F32)
            for ct in range(2):
                ps = ppsu.tile([128, 9], F32, tag="ut", bufs=1)
                for kt in range(18):
                    nc.tensor.matmul(ps, wt1_sb[:125, kt, ct, :],
                                     tfT[:125, kt, :],
                                     start=(kt == 0), stop=(kt == 17))
                nc.vector.tensor_copy(ut1p[:, ct, :], ps)
            ut1i = dram.tile([256, 9], F32, tag="ut1i")
            ut1o = dram.tile([256, 9], F32, tag="ut1o", addr_space="Shared")
            nc.sync.dma_start(
                out=dap(ut1i, 0, [(9, 128), (128 * 9, 2), (1, 9)]), in_=ut1p)
            nc.gpsimd.collective_compute(
                "AllReduce", mybir.AluOpType.add, replica_groups=RG,
                ins=[ut1i.opt()], outs=[ut1o.opt()])
            ut1r = pu.tile([128, 2, 9], F32)
            nc.sync.dma_start(
                out=ut1r, in_=dap(ut1o, 0, [(9, 128), (128 * 9, 2), (1, 9)]))
            ut1a = pu.tile([128, 2, 9], F32)
            for ct in range(2):
                nc.scalar.activation(ut1a[:, ct, :], ut1r[:, ct, :], RELU,
                                     bias=bt1_sb[:, ct:ct + 1])
            utT = pu.tile([128, 9], F32)
            psu = ppsu.tile([128, 9], F32, tag="ut", bufs=1)
            for ct in range(2):
                nc.tensor.matmul(psu, wt2_sb[:, ct, :], ut1a[:, ct, :],
                                 start=(ct == 0), stop=(ct == 1))
            nc.scalar.activation(utT, psu, RELU, bias=bt2_sb)

            # spatial MLP (own nodes)
            us1T = pu.tile([128, 2, 576], F32)
            rhs_li = liT[:125].rearrange("p mt n t -> p mt (n t)")
            for ct in range(2):
                for ch in range(2):
                    ps = ppsu.tile([128, 288], F32, tag="us", bufs=2)
                    for mt in range(4):
                        nc.tensor.matmul(
                            ps, ws1_sb[:125, mt, ct, :],
                            rhs_li[:, mt, 288 * ch:288 * (ch + 1)],
                            start=(mt == 0), stop=(mt == 3))
                    nc.scalar.activation(us1T[:, ct, 288 * ch:288 * (ch + 1)],
                                         ps, RELU, bias=bs1_sb[:, ct:ct + 1])
            usT = pu.tile([128, 576], F32)
            for ch in range(2):
                ps = ppsu.tile([128, 288], F32, tag="us", bufs=2)
                for ct in range(2):
                    nc.tensor.matmul(ps, ws2_sb[:, ct, :],
                                     us1T[:, ct, 288 * ch:288 * (ch + 1)],
                                     start=(ct == 0), stop=(ct == 1))
                nc.scalar.activation(usT[:, 288 * ch:288 * (ch + 1)], ps, RELU,
                                     bias=bs2_sb)
            uT_own = pu.tile([128, 9, 64], F32)   # t-major: [f, t, n]
            usT_v = usT.rearrange("p (n t) -> p n t", t=9)
            for ti in range(9):
                nc.vector.tensor_scalar(
                    out=uT_own[:, ti, :], in0=usT_v[:, :, ti],
                    scalar1=utT[:, ti:ti + 1], scalar2=None,
                    op0=mybir.AluOpType.add)

            # U AllGather (f-major: rows (c,f), cols (t,n))
            uagi = dram.tile([128, 576], F32, tag="uagi")
            uago = dram.tile([NC * 128, 576], F32, tag="uago",
                             addr_space="Shared")
            nc.sync.dma_start(
                out=dap(uagi, 0, [(576, 128), (64, 9), (1, 64)]),
                in_=uT_own)
            nc.gpsimd.collective_compute(
                "AllGather", mybir.AluOpType.bypass, replica_groups=RG,
                ins=[uagi.opt()], outs=[uago.opt()])
            if debug:
                nc.sync.dma_start(out=dbg["dbg_u"][:, :], in_=uago[:, :])
            u2_sb = pu.tile([128, 5, 512], F32)
            for ti in range(5):
                nc.sync.dma_start(
                    out=u2_sb[:, ti, :].rearrange("p (c n) -> p c n", c=8),
                    in_=dap(uago, (2 + ti) * 64,
                            [(576, 128), (128 * 576, 8), (1, 64)]))

            # UB^T own
            ubT = pu.tile([128, 9, 64], F32)
            for ti in range(9):
                ps = ppsu.tile([128, 64], F32, tag="ub", bufs=1)
                nc.tensor.matmul(ps, b_sb, uT_own[:, ti, :],
                                 start=True, stop=True)
                nc.vector.tensor_copy(ubT[:, ti, :], ps)

            # adjacency pairs
            for (i1, i2) in PAIRS:
                pid = PAIR_ID[(i1, i2)]
                ti1 = T9IDX[i1]
                t2 = i2 - 28
                sps = ppsu.tile([128, 4, 64], F32, tag="spair", bufs=2)
                for mc, (m0, cnt) in enumerate(MCH):
                    nc.tensor.matmul(sps[:cnt, mc, :],
                                     u2_sb[:, t2, m0:m0 + cnt],
                                     ubT[:, ti1, :], start=True, stop=True)
                msk = pue.tile([128, 4, 64], F32, tag="msk")
                e_sb = pue.tile([128, 4, 64], F32, tag="esb")
                for mc, (m0, cnt) in enumerate(MCH):
                    nc.vector.tensor_scalar(
                        out=msk[:cnt, mc, :], in0=sps[:cnt, mc, :],
                        scalar1=0.05, scalar2=None, op0=mybir.AluOpType.is_ge)
                    nc.vector.tensor_tensor(
                        out=msk[:cnt, mc, :], in0=msk[:cnt, mc, :],
                        in1=sps[:cnt, mc, :], op=mybir.AluOpType.mult)
                    nc.scalar.activation(e_sb[:cnt, mc, :], msk[:cnt, mc, :],
                                         EXP)
                cps = ppsu.tile([1, 64], F32, tag="cs", bufs=1)
                for mc, (m0, cnt) in enumerate(MCH):
                    nc.tensor.matmul(cps, ones_c[:cnt, :], e_sb[:cnt, mc, :],
                                     start=(mc == 0), stop=(mc == 3))
                rcp = pue.tile([1, 64], F32, tag="rcp")
                nc.vector.reciprocal(rcp, cps)
                rbp = ppsu.tile([128, 64], F32, tag="rb", bufs=1)
                nc.tensor.matmul(rbp, ones_r, rcp, start=True, stop=True)
                for mc, (m0, cnt) in enumerate(MCH):
                    nc.vector.tensor_tensor(
                        out=at_sb[:cnt, pid, mc, :], in0=e_sb[:cnt, mc, :],
                        in1=rbp[:cnt, :], op=mybir.AluOpType.mult)
            if debug:
                at32 = pue.tile([128, 25 * 4 * 64], F32, tag="at32", bufs=1)
                nc.vector.tensor_copy(
                    at32, at_sb.rearrange("p a b c -> p (a b c)"))
                nc.sync.dma_start(out=dbg["dbg_at"][:, :], in_=at32)

        # =============== FC3 resident weights ===============
        pw3 = ctx.enter_context(tc.tile_pool(name="pw3", bufs=1))
        w3_sb = pw3.tile([128, 8, W3RES_J, 128], F16)
        for sl in range(4):
            j0 = sl * 11
            nc.gpsimd.dma_start(
                out=w3_sb[:, :, j0:j0 + 11, :],
                in_=dap(w3s, j0 * 128,
                        [(8192, 128), (128 * 8192, 8), (128, 11), (1, 128)]))

        # =============== passes ===============
        ptr = ctx.enter_context(tc.tile_pool(name="ptr", bufs=2))
        pps = ctx.enter_context(tc.tile_pool(name="pps", bufs=1, space="PSUM"))

        h1i = [dram.tile([128, R_PASS[p]], F32, name=f"h1i{p}", tag=f"h1i{p}")
               for p in range(4)]
        h1o = [dram.tile([FC1W, R_PASS[p]], F32, name=f"h1o{p}", tag=f"h1o{p}",
                         addr_space="Shared") for p in range(4)]
        h2i = [dram.tile([128, R_PASS[p]], F32, name=f"h2i{p}", tag=f"h2i{p}")
               for p in range(4)]
        h2o = [dram.tile([FC2W, R_PASS[p]], F32, name=f"h2o{p}", tag=f"h2o{p}",
                         addr_space="Shared") for p in range(4)]
        # X AllGather: f-major rows (c,f), cols (r,j)
        xagi = [dram.tile([128, R_PASS[p] * 64], F32, name=f"xagi{p}",
                          tag=f"xagi{p}") for p in range(4)]
        xago = [dram.tile([NC * 128, R_PASS[p] * 64], F32, name=f"xago{p}",
                          tag=f"xago{p}", addr_space="Shared")
                for p in range(4)]
        zagi = [dram.tile([R_PASS[p + 1], 4096], F32, name=f"zagi{p}",
                          tag=f"zagi{p}") for p in range(3)]
        zago = [dram.tile([NC * R_PASS[p + 1], 4096], F32, name=f"zago{p}",
                          tag=f"zago{p}", addr_space="Shared")
                for p in range(3)]

        for p in range(4):
            R = R_PASS[p]
            # ---- x^T tiles [k%128, r, kt] ----
            if p == 0:
                xT = ptr.tile([128, 8, KT1], F16, tag="xT", bufs=1)
                nc.gpsimd.dma_start(
                    out=xT[:, :7, :],
                    in_=dap(obs7t, 0, [(7 * KT1, 128), (KT1, 7), (1, KT1)]))
                nc.vector.memset(xT[:, 7, :], 0.0)
            else:
                # PE-transpose zago rows into x^T
                xT = ptr.tile([128, 8, 256], F16, tag="xT", bufs=1,
                              name=f"xT{p}")
                for r in range(R):
                    for c in range(8):
                        stg = ptr.tile([32, 128], F16, tag="xstg",
                                       name=f"xstg{p}_{r}_{c}")
                        nc.gpsimd.dma_start(
                            out=stg,
                            in_=dap(zago[p - 1], (c * R + r) * 4096,
                                    [(128, 32), (1, 128)]))
                        tp = pps.tile([128, 32], F16, tag="tp", bufs=2,
                                      name=f"tp{p}_{r}_{c}")
                        nc.tensor.transpose(tp, stg, ident[:32, :32])
                        nc.vector.tensor_copy(xT[:, r, 32 * c:32 * (c + 1)],
                                              tp)
            # ---- FC1 ----
            ps1 = pps.tile([128, 8], F32, tag="fcA", bufs=1)
            for kt in range(KT1):
                nc.tensor.matmul(ps1[:, :R], w1_sb[:, kt, :], xT[:, :R, kt],
                                 start=(kt == 0), stop=(kt == KT1 - 1))
            h1own = ptr.tile([128, 8], F32, tag="h1own")
            nc.scalar.activation(h1own[:, :R], ps1[:, :R], RELU, bias=b1_sb)
            nc.sync.dma_start(out=h1i[p][:, :], in_=h1own[:, :R])
            nc.gpsimd.collective_compute(
                "AllGather", mybir.AluOpType.bypass, replica_groups=RG,
                ins=[h1i[p].opt()], outs=[h1o[p].opt()])
            h1T = ptr.tile([128, 8, 8], F16, tag="h1T")
            nc.gpsimd.dma_start(
                out=h1T[:, :, :R],
                in_=dap(h1o[p], 0, [(R, 128), (128 * R, 8), (1, R)]))
            if debug and p == 0:
                nc.sync.dma_start(out=dbg["dbg_h1"][:, :], in_=h1o[0][:, :])
            # ---- FC2 ----
            ps2 = pps.tile([128, 8], F32, tag="fcA", bufs=1)
            for kt in range(8):
                nc.tensor.matmul(ps2[:, :R], w2_sb[:, kt, :], h1T[:, kt, :R],
                                 start=(kt == 0), stop=(kt == 7))
            h2own = ptr.tile([128, 8], F32, tag="h2own")
            nc.scalar.activation(h2own[:, :R], ps2[:, :R], RELU, bias=b2_sb)
            nc.sync.dma_start(out=h2i[p][:, :], in_=h2own[:, :R])
            nc.gpsimd.collective_compute(
                "AllGather", mybir.AluOpType.bypass, replica_groups=RG,
                ins=[h2i[p].opt()], outs=[h2o[p].opt()])
            h2T = ptr.tile([128, 8, 8], F16, tag="h2T")
            nc.gpsimd.dma_start(
                out=h2T[:, :, :R],
                in_=dap(h2o[p], 0, [(R, 128), (128 * R, 8), (1, R)]))
            # ---- FC3 ----
            xstage = ptr.tile([128, 8, 64], F32, tag="xstage", bufs=1,
                              name=f"xstage{p}")   # [f, r, j]
            w3t = None
            for j in range(64):
                if j < W3RES_J:
                    wblk = lambda kt, j=j: w3_sb[:, kt, j, :]
                else:
                    if (j - W3RES_J) % 4 == 0:
                        w3t = ptr.tile([128, 8, 4, 128], F16, tag="w3t",
                                       name=f"w3t{p}_{j}")
                        nc.gpsimd.dma_start(
                            out=w3t,
                            in_=dap(w3s, j * 128,
                                    [(8192, 128), (128 * 8192, 8),
                                     (128, 4), (1, 128)]))
                    wblk = (lambda kt, j=j, w3t=w3t:
                            w3t[:, kt, (j - W3RES_J) % 4, :])
                ps3 = pps.tile([128, 8], F32, tag="fc3", bufs=2,
                               name=f"ps3_{p}_{j}")
                for kt in range(8):
                    nc.tensor.matmul(ps3[:, :R], wblk(kt), h2T[:, kt, :R],
                                     start=(kt == 0), stop=(kt == 7))
                nc.scalar.activation(xstage[:, :R, j], ps3[:, :R], RELU,
                                     bias=b3_sb[:, j:j + 1])
            # ---- X AllGather (f-major) ----
            nc.sync.dma_start(
                out=dap(xagi[p], 0, [(R * 64, 128), (64, R), (1, 64)]),
                in_=xstage[:, :R, :])
            nc.gpsimd.collective_compute(
                "AllGather", mybir.AluOpType.bypass, replica_groups=RG,
                ins=[xagi[p].opt()], outs=[xago[p].opt()])
            if debug and p == 0:
                nc.sync.dma_start(out=dbg["dbg_x0"][:, :], in_=xago[0][:, :])

            # ---- Xk^T tiles [f, c, n] ----
            need = sorted({(t - k) % L for t in CONV_TS[p] for k in range(3)})
            xkT = {}
            for tv in need:
                sp, slot = XROW[p][tv]
                if sp == 0 and slot == 7 and p != 0:
                    xkT[tv] = zrow_sb
                    continue
                dst = ptr.tile([128, 8, 64], F16, tag=f"xk{tv % 4}",
                               name=f"xk_{p}_{tv}")
                Rs = R_PASS[sp]
                nc.gpsimd.dma_start(
                    out=dst,
                    in_=dap(xago[sp], slot * 64,
                            [(Rs * 64, 128), (128 * Rs * 64, 8), (1, 64)]))
                xkT[tv] = dst
            if p == 0:
                nc.gpsimd.dma_start(
                    out=zrow_sb,
                    in_=dap(xago[0], 7 * 64,
                            [(8 * 64, 128), (128 * 8 * 64, 8), (1, 64)]))

            # ---- conv units ----
            for t in CONV_TS[p]:
                relu_parts = []
                for k in range(3):
                    tv = (t - k) % L
                    xk = xkT[tv].rearrange("p c n -> p (c n)")
                    if k == 0:
                        dirs = [(PAIR_ID[(t, t)], 0, 64)]
                        wsl = wfb_sb[:, 0, :]
                        ow = 64
                    else:
                        dirs = [(PAIR_ID[((t - k) % L, t)], 0, 64),
                                (PAIR_ID[((t + k) % L, t)], 64, 128)]
                        wsl = wfb_sb.rearrange("p d o -> p (d o)")[
                            :, (2 * k - 1) * 64:(2 * k + 1) * 64]
                        ow = 128
                    y_ps = pps.tile([128, 4, 128], F32, tag="yps", bufs=2,
                                    name=f"yps{p}_{t}_{k}")
                    y_sb = ptr.tile([128, 4, 128], F16, tag="ysb",
                                    name=f"ysb{p}_{t}_{k}")
                    for mc, (m0, cnt) in enumerate(MCH):
                        nc.tensor.matmul(y_ps[:cnt, mc, :ow],
                                         xk[:, m0:m0 + cnt],
                                         wsl, start=True, stop=True)
                        nc.vector.tensor_copy(y_sb[:cnt, mc, :ow],
                                              y_ps[:cnt, mc, :ow])
                    hps = pps.tile([64, 64], F32, tag="hps", bufs=1,
                                   name=f"hps{p}_{t}_{k}")
                    n_mm = len(dirs) * 4
                    i_mm = 0
                    for mc, (m0, cnt) in enumerate(MCH):
                        for (pid, o0, o1) in dirs:
                            nc.tensor.matmul(
                                hps, at_sb[:cnt, pid, mc, :],
                                y_sb[:cnt, mc, o0:o1],
                                start=(i_mm == 0), stop=(i_mm == n_mm - 1))
                            i_mm += 1
                    hb = ptr.tile([64, 64], F32, tag="hb",
                                  name=f"hb{p}_{t}_{k}")
                    nc.vector.tensor_tensor(out=hb, in0=hps, in1=bcb_sb,
                                            op=mybir.AluOpType.add)
                    rk = ptr.tile([64, 64], F32, tag=f"rk{k}",
                                  name=f"rk{p}_{t}_{k}")
                    nc.scalar.activation(rk, hb, RELU)
                    relu_parts.append(rk)
                zacc = ptr.tile([64, 64], F32, tag="zacc",
                                name=f"zacc{p}_{t}")
                nc.vector.tensor_tensor(out=zacc, in0=relu_parts[0],
                                        in1=relu_parts[1],
                                        op=mybir.AluOpType.add)
                nc.vector.tensor_tensor(out=zacc, in0=zacc,
                                        in1=relu_parts[2],
                                        op=mybir.AluOpType.add)
                if t == 32:
                    nc.sync.dma_start(
                        out=dap(out_ext, p * NODES_PER_CORE * DO,
                                [(64, 64), (1, 64)]),
                        in_=zacc)
                if p < 3:
                    r = CONV_TS[p].index(t)
                    nc.sync.dma_start(
                        out=dap(zagi[p], r * 4096, [(64, 64), (1, 64)]),
                        in_=zacc)
            if p < 3:
                nc.gpsimd.collective_compute(
                    "AllGather", mybir.AluOpType.bypass, replica_groups=RG,
                    ins=[zagi[p].opt()], outs=[zago[p].opt()])

    nc.finalize()
    return nc


# ======================= host side =======================
_NC_CACHE = {}


def _get_nc(debug=False):
    if debug not in _NC_CACHE:
        _NC_CACHE[debug] = build(debug)
    return _NC_CACHE[debug]


def make_in_maps(inputs):
    obs = np.asarray(inputs["observation"], np.float32)
    tf = np.asarray(inputs["time_feats"], np.float32)
    lin = np.asarray(inputs["layer_initial"], np.float32)
    Wf = np.asarray(inputs["Wf"], np.float32)
    Wb = np.asarray(inputs["Wb"], np.float32)
    wfb = np.ascontiguousarray(
        np.stack([Wf[0] + Wb[0], Wf[1], Wb[1], Wf[2], Wb[2]]))
    w3 = np.asarray(inputs["Wfc3"], np.float32)
    b3 = np.asarray(inputs["bfc3"], np.float32)
    o7 = obs[26:33]                                   # (7, 32000)
    obs7t = np.ascontiguousarray(
        o7.reshape(7, KT1, 128).transpose(2, 0, 1))   # (128, 7, 250)
    kfn = KTF // NC
    in_maps = []
    for c in range(NC):
        n0, cnt = NODE0[c], REAL_NODES[c]
        li = np.zeros((N, NODES_PER_CORE, 9), np.float32)
        li[:, :cnt, :] = lin[n0:n0 + cnt][:, T9, :].transpose(2, 0, 1)
        w3s = np.zeros((FC2W, 8192), np.float32)
        b3s = np.zeros((8192,), np.float32)
        c0, c1 = 8192 * c, min(8192 * (c + 1), 64000)
        w3s[:, :c1 - c0] = w3[:, c0:c1]
        b3s[:c1 - c0] = b3[c0:c1]
        b3st = np.ascontiguousarray(b3s.reshape(64, 128).T)   # [p, j]
        kf0 = kfn * c
        in_maps.append({
            "li": li,
            "tfs": np.ascontiguousarray(tf[T9][:, kf0:kf0 + kfn].T),
            "obs7t": obs7t,
            "ws1": np.asarray(inputs["Ws1"], np.float32),
            "bs1": np.asarray(inputs["bs1"], np.float32),
            "ws2": np.asarray(inputs["Ws2"], np.float32),
            "bs2": np.asarray(inputs["bs2"], np.float32),
            "wt1s": np.ascontiguousarray(
                np.asarray(inputs["Wt1"], np.float32)[kf0:kf0 + kfn]),
            "bt1": np.asarray(inputs["bt1"], np.float32),
            "wt2": np.asarray(inputs["Wt2"], np.float32),
            "bt2": np.asarray(inputs["bt2"], np.float32),
            "bmat": np.asarray(inputs["B"], np.float32),
            "w1s": np.ascontiguousarray(
                np.asarray(inputs["Wfc1"], np.float32)[:, 128 * c:128 * (c + 1)]),
            "b1s": np.ascontiguousarray(
                np.asarray(inputs["bfc1"], np.float32)[128 * c:128 * (c + 1)]),
            "w2s": np.ascontiguousarray(
                np.asarray(inputs["Wfc2"], np.float32)[:, 128 * c:128 * (c + 1)]),
            "b2s": np.ascontiguousarray(
                np.asarray(inputs["bfc2"], np.float32)[128 * c:128 * (c + 1)]),
            "w3s": w3s,
            "b3st": b3st,
            "wfb": wfb,
            "bconv": np.asarray(inputs["bconv"], np.float32),
        })
    return in_maps


def _assemble(results):
    out = np.zeros((4, N, DO), np.float32)
    for c in range(NC):
        n0, cnt = NODE0[c], REAL_NODES[c]
        out[:, n0:n0 + cnt, :] = results[c]["out"][:, :cnt, :]
    return out


def kernel(**inputs):
    nc = _get_nc(debug=False)
    in_maps = make_in_maps(inputs)
    res = run_bass_kernel_spmd(nc, in_maps, core_ids=list(range(NC)))
    return _assemble(res.results)
